# revision 12
# baseline (speedup 1.0000x reference)
"""Trainium2 Bass kernel for GNN mean-aggregation message passing.

  m = relu(concat(y[src], ex) @ W1.T + b1)        per edge
  z = segment_mean(m, dst)                        per node (0 for isolated)
  h = relu(z @ W2.T + b2)                         per node

Strategy (8 NeuronCores, one SPMD program, edge-parallel by dst range):
  - Host shards edges by dst node range (N/8 nodes per core) and sorts each
    core's edges by (dst-window, dst). Per-window tile counts are unified
    across cores (max), so a single program fits all shards; padding slots
    carry all-zero features and zero one-hot scatter weight.
  - Host materializes per-edge features featT[49, slot] = [y[src]; ex; 1]
    in bf16, edge-slot order. The device runs the whole MLP + aggregation:
    per 128-edge tile, m = relu(featT_tile.T @ W1b) on PE+ACT, then a
    scatter one-hot matmul s.T[48, win] += m.T @ O accumulates the segment
    sum in PSUM (O is 0/1 in fp8, exact). dst-sorted tiles make O spans
    tiny (~10 cols).
  - Window drain: z = s * (1/deg) (DVE), h.T = relu(W2.T @ z.T + b2)
    (PE+ACT), DMA out. Mean uses host-precomputed 1/deg (0-deg -> 0 via
    zero sums).
"""

import os

import numpy as np
import ml_dtypes

# timing-ablation knob (empty in production): subset of
# {"w1", "act", "scatter", "feat", "drain"}
DISABLE = set(os.environ.get("KDISABLE", "").split(",")) - {""}

N_CORES = 8
WIN = 1024         # nodes per PSUM scatter window (2 banks)
TILE_E = 128       # edges per tile (PE contraction dim for scatter)
SUPER = 8          # tiles per PSUM-m batch / ACT relu batch

BF16 = ml_dtypes.bfloat16
REPEAT = 1  # run the body N times (timing experiments only)


def _preprocess(y, ex, W1, b1, W2, b2, src, dst):
    N, ND = y.shape
    E, ED = ex.shape
    D = ND + ED
    K = D + 1  # feature rows incl. bias-ones row
    NPC = N // N_CORES
    NW = (NPC + WIN - 1) // WIN

    cnt = np.bincount(dst, minlength=N)
    inv_cnt = (1.0 / np.maximum(cnt, 1)).astype(np.float32)

    core_of = (dst // NPC).astype(np.int64)
    win_of = ((dst - core_of * NPC) // WIN).astype(np.int64)
    cw = core_of * NW + win_of
    key = cw * np.int64(N + 1) + dst
    order = np.argsort(key, kind="stable")

    dst_s = dst[order]
    src_s = src[order]
    ex_s = ex[order]
    core_s = core_of[order]
    win_s = win_of[order]
    cw_s = cw[order]

    cw_cnt = np.bincount(cw_s, minlength=N_CORES * NW).reshape(N_CORES, NW)
    # tiles per window, rounded to even (W1 matmuls process tile PAIRS)
    T_w = 2 * ((cw_cnt.max(axis=0) + 2 * TILE_E - 1) // (2 * TILE_E))  # [NW]
    win_block_base = np.concatenate([[0], np.cumsum(T_w)])
    B_tot = int(win_block_base[-1])
    E_slots = B_tot * TILE_E

    # rank of each edge within its (core, window) run
    cw_start = np.zeros(N_CORES * NW + 1, np.int64)
    cw_start[1:] = np.cumsum(cw_cnt.reshape(-1))
    rank = np.arange(E, dtype=np.int64) - cw_start[cw_s]
    slot = win_block_base[win_s] * TILE_E + rank
    tile_of = slot // TILE_E
    p_in_tile = slot % TILE_E

    # per-tile dst span (window-relative), unioned over cores
    rel = dst_s - core_s * NPC - win_s * WIN
    lo_t = np.full(B_tot, np.int64(1 << 60))
    hi_t = np.full(B_tot, np.int64(-1))
    np.minimum.at(lo_t, tile_of, rel)
    np.maximum.at(hi_t, tile_of, rel)
    empty = hi_t < 0
    lo_t[empty] = 0
    hi_t[empty] = 0
    span_t = hi_t - lo_t + 1
    col_off = np.concatenate([[0], np.cumsum(span_t)])
    C_tot = int(col_off[-1])
    o_col = col_off[tile_of] + (rel - lo_t[tile_of])

    # K-packed features: tile pair (2p, 2p+1) stacked into 98 rows so one
    # matmul with a block-diagonal [98, 96] weight computes both tiles' m.
    # Column c of pair p carries edge slots 2p*128+c (rows 0:49) and
    # (2p+1)*128+c (rows 49:98).
    half = slot // TILE_E % 2
    pcol = (slot // (2 * TILE_E)) * TILE_E + slot % TILE_E
    featT = np.zeros((N_CORES, 2 * K, E_slots // 2), BF16)
    O_a = np.zeros((N_CORES, TILE_E, C_tot), ml_dtypes.float8_e4m3)
    y_bf = y.astype(BF16)
    ex_bf = ex_s.astype(BF16)
    for c in range(N_CORES):
        m = core_s == c
        base = half[m] * K
        pc = pcol[m]
        for r in range(ND):
            featT[c, base + r, pc] = y_bf[src_s[m], r]
        for r in range(ED):
            featT[c, base + ND + r, pc] = ex_bf[m, r]
        featT[c, base + D, pc] = 1.0
        O_a[c, p_in_tile[m], o_col[m]] = 1.0

    cinv = np.empty((N_CORES, D, NPC), np.float32)
    for c in range(N_CORES):
        cinv[c] = np.broadcast_to(inv_cnt[c * NPC : (c + 1) * NPC], (D, NPC))

    meta = {
        "N": N, "E": E, "ND": ND, "ED": ED, "D": D, "K": K, "NPC": NPC,
        "n_win": NW, "T_w": T_w, "win_block_base": win_block_base,
        "B_tot": B_tot, "E_slots": E_slots, "C_tot": C_tot,
        "lo_t": lo_t, "span_t": span_t, "col_off": col_off,
    }
    w1b = np.concatenate([W1.T, b1[None, :]], 0).astype(BF16)      # [49, 48]
    w1b2 = np.zeros((2 * K, 2 * D), BF16)                          # [98, 96]
    w1b2[:K, :D] = w1b
    w1b2[K:, D:] = w1b
    consts = dict(
        W1b2=w1b2,
        W2b=np.ascontiguousarray(W2.T).astype(np.float32),         # [48, 32]
        b2=np.ascontiguousarray(b2.reshape(-1, 1)).astype(np.float32),
    )
    per_core = dict(featT=featT, O=O_a, cinv=cinv)
    return consts, per_core, meta


def _split_excess_waits(nc, mybir):
    """This walrus build accepts at most 1 sync wait per instruction (0 on
    Drain). Move extras onto NOPs inserted just before, same engine."""
    for fn in nc.m.functions:
        for bb in fn.blocks:
            new_list = []
            for ins in bb.instructions:
                si = ins.sync_info
                limit = 0 if isinstance(ins, mybir.InstDrain) else 1
                if si is not None and si.on_wait and len(si.on_wait) > limit:
                    waits = list(si.on_wait)
                    keep, extra = waits[:limit], waits[limit:]
                    while extra:
                        chunk, extra = extra[:1], extra[1:]
                        nop = mybir.InstNoOp(
                            name=nc.get_next_instruction_name(), ins=[], outs=[])
                        nop.engine = ins.engine
                        nop.sync_info = mybir.SyncInfo(on_wait=chunk, on_update=[])
                        nc.register_instruction(nop)
                        new_list.append(nop)
                    si.on_wait = keep
                new_list.append(ins)
            bb.instructions[:] = new_list


def _build_program(meta):
    import concourse.bacc as bacc
    import concourse.mybir as mybir
    import concourse.tile as tile

    f32 = mybir.dt.float32
    bf16 = mybir.dt.bfloat16
    f8 = mybir.dt.float8e4
    Relu = mybir.ActivationFunctionType.Relu
    MULT = mybir.AluOpType.mult

    D, K, NPC, NW = meta["D"], meta["K"], meta["NPC"], meta["n_win"]
    T_w, wbb = meta["T_w"], meta["win_block_base"]
    E_slots, C_tot = meta["E_slots"], meta["C_tot"]
    lo_t, span_t, col_off = meta["lo_t"], meta["span_t"], meta["col_off"]
    OD = 32

    nc = bacc.Bacc("TRN2")
    feat_ext = nc.dram_tensor("featT", [K, E_slots], bf16, kind="ExternalInput")
    O_ext = nc.dram_tensor("Omat", [TILE_E, C_tot], f8, kind="ExternalInput")
    cinv_ext = nc.dram_tensor("cinv", [D, NPC], f32, kind="ExternalInput")
    w1b_ext = nc.dram_tensor("W1b", [K, D], bf16, kind="ExternalInput")
    w2b_ext = nc.dram_tensor("W2b", [D, OD], f32, kind="ExternalInput")
    b2_ext = nc.dram_tensor("b2", [OD, 1], f32, kind="ExternalInput")
    out_ext = nc.dram_tensor("hT", [OD, NPC], f32, kind="ExternalOutput")

    with tile.TileContext(nc) as tc:
        with (
            tc.tile_pool(name="const", bufs=1) as cpool,
            tc.tile_pool(name="io", bufs=2) as iopool,
            tc.tile_pool(name="msb", bufs=4) as mpool,
            tc.tile_pool(name="psM", bufs=2, space="PSUM") as psM,
            tc.tile_pool(name="psZ", bufs=2, space="PSUM") as psZ,
            tc.tile_pool(name="psH", bufs=1, space="PSUM") as psH,
        ):
            w1b_sb = cpool.tile([K, D], bf16)
            nc.sync.dma_start(out=w1b_sb[:], in_=w1b_ext[:])
            w2b_sb = cpool.tile([D, OD], f32)
            nc.sync.dma_start(out=w2b_sb[:], in_=w2b_ext[:])
            b2_sb = cpool.tile([OD, 1], f32)
            nc.sync.dma_start(out=b2_sb[:], in_=b2_ext[:])
            zl_bf = cpool.tile([1, D], bf16)
            nc.any.memset(zl_bf[:], 0)
            zr_bf = cpool.tile([1, 512], bf16)
            nc.any.memset(zr_bf[:], 0)

            for _rep in range(REPEAT):
                for w in range(NW):
                    T = int(T_w[w])
                    if T == 0:
                        continue
                    wn = min(WIN, NPC - w * WIN)
                    b0 = int(wbb[w])
                    e0 = b0 * TILE_E
                    gn = T * TILE_E
                    c0 = int(col_off[b0])
                    cn = int(col_off[b0 + T]) - c0

                    feat_t = iopool.tile([K, gn], bf16, tag="feat")
                    if "feat" not in DISABLE:
                        nc.sync.dma_start(
                            out=feat_t[:], in_=feat_ext[:, e0 : e0 + gn])
                    o_t = iopool.tile([TILE_E, cn], f8, tag="omat")
                    nc.sync.dma_start(out=o_t[:], in_=O_ext[:, c0 : c0 + cn])
                    cinv_t = iopool.tile([D, WIN], f32, tag="cinv")
                    nc.sync.dma_start(
                        out=cinv_t[:, :wn],
                        in_=cinv_ext[:, w * WIN : w * WIN + wn])

                    psz = psZ.tile([D, WIN], f32, tag="psz")
                    for j in range(0, WIN, 512):
                        nc.tensor.matmul(
                            psz[:, j : j + 512], zl_bf[:], zr_bf[:],
                            start=True, stop=True)

                    def emit_scatter(m_sb, s, sb, final):
                        for t in range(sb):
                            bt = b0 + s + t
                            lo = int(lo_t[bt])
                            sp = int(span_t[bt])
                            off = int(col_off[bt]) - c0
                            last = final and t == sb - 1
                            # split at 512-col PSUM bank boundary
                            cuts = [lo, sp]
                            if lo // 512 != (lo + sp - 1) // 512:
                                sp1 = (lo // 512 + 1) * 512 - lo
                                cuts = [lo, sp1, lo + sp1, sp - sp1]
                            for k in range(0, len(cuts), 2):
                                clo, csp = cuts[k], cuts[k + 1]
                                if csp <= 0:
                                    continue
                                nc.tensor.matmul(
                                    psz[:, clo : clo + csp],
                                    m_sb[:, t * D : (t + 1) * D],
                                    o_t[:, off + (clo - lo)
                                        : off + (clo - lo) + csp],
                                    start=False,
                                    stop=last and k + 2 >= len(cuts),
                                    skip_group_check=True)

                    # software-pipelined: scatter of super s is emitted after
                    # the W1 matmuls of super s+1, hiding the ACT relu latency
                    n_super = (T + SUPER - 1) // SUPER
                    pending = None
                    for si in range(n_super):
                        s = si * SUPER
                        sb = min(SUPER, T - s)
                        ps_m = psM.tile([TILE_E, SUPER * D], f32, tag="psm")
                        for t in range(sb if "w1" not in DISABLE else 0):
                            nc.tensor.matmul(
                                ps_m[:, t * D : (t + 1) * D],
                                feat_t[:, (s + t) * TILE_E
                                       : (s + t + 1) * TILE_E],
                                w1b_sb[:], start=True, stop=True)
                        m_sb = mpool.tile([TILE_E, SUPER * D], bf16, tag="m")
                        if "act" not in DISABLE and "w1" not in DISABLE:
                            nc.scalar.activation(
                                out=m_sb[:, : sb * D], in_=ps_m[:, : sb * D],
                                func=Relu)
                        if pending is not None and "scatter" not in DISABLE:
                            emit_scatter(*pending, final=False)
                        pending = (m_sb, s, sb)
                    if "scatter" not in DISABLE:
                        emit_scatter(*pending, final=True)

                    zt = mpool.tile([D, WIN], f32, tag="zt")
                    nc.vector.tensor_tensor(
                        out=zt[:, :wn], in0=psz[:, :wn], in1=cinv_t[:, :wn],
                        op=MULT)
                    ps_h = psH.tile([OD, WIN], f32, tag="psh")
                    for j in range(0, wn, 512):
                        jn = min(512, wn - j)
                        nc.tensor.matmul(
                            ps_h[:, j : j + jn], w2b_sb[:], zt[:, j : j + jn],
                            start=True, stop=True)
                    h_sb = mpool.tile([OD, WIN], f32, tag="h")
                    nc.scalar.activation(
                        out=h_sb[:, :wn], in_=ps_h[:, :wn], func=Relu,
                        bias=b2_sb[:, 0:1])
                    nc.sync.dma_start(
                        out=out_ext[:, w * WIN : w * WIN + wn],
                        in_=h_sb[:, :wn])

    nc.compile()
    _split_excess_waits(nc, mybir)
    return nc


def build_in_maps(consts, per_core):
    in_maps = []
    for c in range(N_CORES):
        in_maps.append({
            "featT": per_core["featT"][c],
            "Omat": per_core["O"][c],
            "cinv": per_core["cinv"][c],
            "W1b": consts["W1b"],
            "W2b": consts["W2b"],
            "b2": consts["b2"],
        })
    return in_maps


def kernel(y, ex, W1, b1, W2, b2, src, dst):
    from concourse.bass_utils import run_bass_kernel_spmd

    y = np.asarray(y, dtype=np.float32)
    ex = np.asarray(ex, dtype=np.float32)
    W1 = np.asarray(W1, dtype=np.float32)
    b1 = np.asarray(b1, dtype=np.float32)
    W2 = np.asarray(W2, dtype=np.float32)
    b2 = np.asarray(b2, dtype=np.float32)
    src = np.asarray(src, dtype=np.int32)
    dst = np.asarray(dst, dtype=np.int32)

    consts, per_core, meta = _preprocess(y, ex, W1, b1, W2, b2, src, dst)
    nc = _build_program(meta)
    in_maps = build_in_maps(consts, per_core)
    res = run_bass_kernel_spmd(nc, in_maps, list(range(N_CORES)))

    NPC = meta["NPC"]
    h = np.empty((meta["N"], 32), dtype=np.float32)
    for c in range(N_CORES):
        h[c * NPC : (c + 1) * NPC, :] = res.results[c]["hT"].T
    return h
